# revision 2
# baseline (speedup 1.0000x reference)
"""CircleLoss Trainium2 kernel (8-core SPMD), v3.0.

Math: for S = cosine-sim(enc, dec) [N,N], both loss directions reduce to
per-wrapped-diagonal logsumexps of one matrix:
    out = mean_{d=1..N-1} softplus(L[d] + lse_p)
    L[d]  = log sum_j exp(g(S[j,(j+d)%N])),  g(s) = GAMMA*(max(s,-M)^2 - M^2)
    lse_p = logsumexp_j h(S[j,j])  (exact, computed on host)
g in [-4, 60] so sum(exp(g)) fits f32 with no max-pass.

Device chain per element: w = sqrt(GAMMA)*s from a bf16 matmul (norms and
sqrt(GAMMA) folded into host-prepped operands), then
    t  = relu(w + 2)          (= max(w,-2)+2, in [0,10], f16)
    v4 = (t - 4) * t          (= max(w,-2)^2 - 4 = g, one fused DVE
                               scalar_tensor_tensor, f16, 2x mode)
    E  = exp(v4)              (ACT, bf16 out)
The PSUM drain that produces t is split between ACT (Relu w+2) and DVE
(dual-op tensor_scalar max -2 then add +2) per-group to balance engine load.

Sharding: core r owns rows [1024r, 1024r+1024). Host pre-normalizes both
embeddings (f64) and ships bf16 transposed operands. Each core computes its
1024 x 8320 sheared slab (row-tile bj reads dec window cols shifted by
128*bj so wrapped diagonals align across tiles: element (p, y) of every
tile has diagonal d = y - p), accumulates 4 row-tiles per quad in bf16,
bounces the two quad stripes through DRAM with a sheared re-read that turns
diagonals into columns, and column-sums via one-hot matmuls in PSUM. Host
sums the 8 per-core [8192] partials and finishes in float64.
"""

import numpy as np
import ml_dtypes

import concourse.bass as bass
import concourse.bacc as bacc
import concourse.mybir as mybir
from concourse.tile import TileContext
from concourse.bass_utils import run_bass_kernel_spmd

N = 8192
D = 128
P = 128
NCORES = 8
R = N // NCORES          # 1024 rows per core
NBJ = R // P             # 8 row-tiles per core
WIN = 9216               # dec window columns per core
W2 = 8320                # sheared slab width (y = x - 128*bj, d = y - p)
NWC = 16                 # 512-wide d-chunks in the output
M_M = 0.25
GAMMA = 64.0
SQG = 8.0                # sqrt(GAMMA), folded into enc operand on host
EPS = 1e-5

# per-bj wide groups over y: 5 x 1536 + 1 x 640
GROUPS = [(0, 1536), (1536, 1536), (3072, 1536), (4608, 1536),
          (6144, 1536), (7680, 640)]
N_SLOTS = 48                     # 8 tiles x 6 groups
N_ACT_DRAIN = 26                 # slots drained via ACT Relu (rest DVE ts)
ACT_SLOTS = frozenset(
    i for i in range(N_SLOTS)
    if (i * N_ACT_DRAIN) // N_SLOTS != ((i + 1) * N_ACT_DRAIN) // N_SLOTS)

F32 = mybir.dt.float32
F16 = mybir.dt.float16
BF16 = mybir.dt.bfloat16

_CACHE = {}


def _build_program():
    nc = bacc.Bacc("TRN2", target_bir_lowering=False, debug=False,
                   num_devices=NCORES)
    encT = nc.dram_tensor("encT", [P, R], BF16, kind="ExternalInput")
    decT = nc.dram_tensor("decT", [P, WIN], BF16, kind="ExternalInput")
    acc_out = nc.dram_tensor("acc_out", [NWC, 512], F32, kind="ExternalOutput")
    stripes = nc.dram_tensor("stripes", [2, P, W2], BF16, kind="Internal")

    mx = mybir.AluOpType.max
    add = mybir.AluOpType.add
    mult = mybir.AluOpType.mult
    AF = mybir.ActivationFunctionType

    with TileContext(nc) as tc:
        with (
            tc.tile_pool(name="persist", bufs=1) as persist,
            tc.tile_pool(name="mm", bufs=2, space="PSUM") as mmp,
            tc.tile_pool(name="apsum", bufs=1, space="PSUM") as apsum,
            tc.tile_pool(name="trow", bufs=2) as trow_pool,
            tc.tile_pool(name="vrow", bufs=2) as vrow_pool,
            tc.tile_pool(name="erow", bufs=3) as erow_pool,
            tc.tile_pool(name="erpool", bufs=4) as erpool,
        ):
            enc_sb = persist.tile([P, R], BF16)
            nc.sync.dma_start(out=enc_sb[:], in_=encT[:, :])
            dec_sb = persist.tile([P, WIN], BF16)
            for dk in range(3):
                nc.sync.dma_start(out=dec_sb[:, dk * 3072:(dk + 1) * 3072],
                                  in_=decT[:, dk * 3072:(dk + 1) * 3072])

            onehot = persist.tile([P, NWC * NWC], BF16)
            bias_p2 = persist.tile([P, 1], F32)
            nc.gpsimd.memset(bias_p2[:], 2.0)
            nc.gpsimd.memset(onehot[:], 0.0)
            for wc in range(NWC):
                nc.gpsimd.memset(onehot[:, wc * NWC + wc:wc * NWC + wc + 1],
                                 1.0)

            q0 = persist.tile([P, W2], BF16)
            q1 = persist.tile([P, W2], BF16)
            quads = [q0, q1]
            acc_ps = apsum.tile([NWC, 512], F32)
            nc.vector.memset(acc_ps[:], 0.0)
            acc_sb = persist.tile([NWC, 512], F32)

            # sheared reads eligible after stripe write of group g completes:
            # read rq covers stripe cols [1024rq, 1024rq+1150]
            reads_after = {0: [0], 1: [1], 2: [2, 3], 3: [4], 4: [5, 6],
                           5: [7]}

            for q in range(2):
                for bjl in range(4):
                    bj = 4 * q + bjl
                    tr = trow_pool.tile([P, W2], F16, tag="tr")
                    vr = vrow_pool.tile([P, W2], F16, tag="vr")
                    er_ = None if bjl == 0 else \
                        erow_pool.tile([P, W2], BF16, tag="erow")
                    for g, (y0, gw) in enumerate(GROUPS):
                        ps = mmp.tile([P, 1536], F32, tag="ps")
                        for c0 in range(0, gw, 512):
                            cw = min(512, gw - c0)
                            nc.tensor.matmul(
                                ps[:, c0:c0 + cw],
                                lhsT=enc_sb[:, bj * P:(bj + 1) * P],
                                rhs=dec_sb[:, 128 * bj + y0 + c0:
                                           128 * bj + y0 + c0 + cw],
                                start=True, stop=True)
                        if (bj * 6 + g) in ACT_SLOTS:
                            nc.scalar.activation(tr[:, y0:y0 + gw],
                                                 ps[:, 0:gw], AF.Relu,
                                                 bias=bias_p2[:, 0:1],
                                                 scale=1.0)
                        else:
                            nc.vector.tensor_scalar(
                                out=tr[:, y0:y0 + gw], in0=ps[:, 0:gw],
                                scalar1=-2.0, scalar2=2.0, op0=mx, op1=add)
                    # v4 = (t - 4) * t = max(w,-2)^2 - 4, one fused DVE op
                    nc.vector.scalar_tensor_tensor(
                        out=vr[:], in0=tr[:], scalar=-4.0, in1=tr[:],
                        op0=add, op1=mult)
                    if bjl == 0:
                        nc.scalar.activation(quads[q][:], vr[:], AF.Exp,
                                             scale=1.0)
                    elif bjl != 3:
                        nc.scalar.activation(er_[:], vr[:], AF.Exp,
                                             scale=1.0)
                        nc.vector.tensor_tensor(out=quads[q][:], in0=er_[:],
                                                in1=quads[q][:], op=add)
                    else:
                        # last tile of the quad: per-group exp+add so the
                        # stripe DMA and sheared re-reads pipeline
                        for g, (y0, gw) in enumerate(GROUPS):
                            nc.scalar.activation(er_[:, y0:y0 + gw],
                                                 vr[:, y0:y0 + gw], AF.Exp,
                                                 scale=1.0)
                            nc.vector.tensor_tensor(
                                out=quads[q][:, y0:y0 + gw],
                                in0=er_[:, y0:y0 + gw],
                                in1=quads[q][:, y0:y0 + gw], op=add)
                            nc.sync.dma_start(
                                out=stripes[q, :, y0:y0 + gw],
                                in_=quads[q][:, y0:y0 + gw])
                            for rq in reads_after[g]:
                                er = erpool.tile([P, 1024], BF16, tag="er")
                                nc.sync.dma_start(
                                    out=er[:],
                                    in_=bass.AP(tensor=stripes,
                                                offset=q * P * W2 + 1024 * rq,
                                                ap=[[W2 + 1, P], [1, 1024]]))
                                for h in range(2):
                                    wc = 2 * rq + h
                                    nc.tensor.matmul(
                                        acc_ps[:],
                                        lhsT=onehot[:, wc * NWC:(wc + 1) * NWC],
                                        rhs=er[:, h * 512:(h + 1) * 512],
                                        start=False, stop=False,
                                        skip_group_check=True)
            nc.scalar.copy(acc_sb[:], acc_ps[:])
            nc.sync.dma_start(out=acc_out[:, :], in_=acc_sb[:])
    nc.compile()
    return nc


def make_in_maps(enc: np.ndarray, dec: np.ndarray):
    """Host prep: normalize in f64, fold sqrt(GAMMA) into enc, transpose,
    cast bf16, build per-core window slices. Returns (in_maps, lse_p)."""
    e64 = enc.astype(np.float64)
    d64 = dec.astype(np.float64)
    en = np.sqrt((e64 * e64).sum(1, keepdims=True))
    dn = np.sqrt((d64 * d64).sum(1, keepdims=True))
    encn8 = (e64 / en * SQG).astype(ml_dtypes.bfloat16)
    decn = (d64 / dn).astype(ml_dtypes.bfloat16)

    s_jj = (e64 * d64).sum(1) / (en[:, 0] * dn[:, 0] + EPS)
    h = -np.maximum(1.0 + M_M - s_jj, 0.0) * (s_jj - (1.0 - M_M)) * GAMMA
    hm = h.max()
    lse_p = hm + np.log(np.exp(h - hm).sum())

    in_maps = []
    for r in range(NCORES):
        idx = (r * R + np.arange(WIN)) % N
        in_maps.append({
            "encT": np.ascontiguousarray(encn8[r * R:(r + 1) * R].T),
            "decT": np.ascontiguousarray(decn[idx].T),
        })
    return in_maps, lse_p


def kernel(encoder_output: np.ndarray, decoder_output: np.ndarray) -> np.ndarray:
    enc = np.ascontiguousarray(encoder_output, dtype=np.float32)
    dec = np.ascontiguousarray(decoder_output, dtype=np.float32)
    assert enc.shape == (N, D) and dec.shape == (N, D)

    if "nc" not in _CACHE:
        _CACHE["nc"] = _build_program()
    nc = _CACHE["nc"]

    in_maps, lse_p = make_in_maps(enc, dec)
    res = run_bass_kernel_spmd(nc, in_maps, core_ids=list(range(NCORES)))

    sum_exp = np.zeros(N, dtype=np.float64)
    for r in range(NCORES):
        acc = res.results[r]["acc_out"].astype(np.float64)      # [NWC, 512]
        sum_exp += acc.reshape(N)                               # d = 512*wc + f
    L = np.log(sum_exp[1:])
    x = L + lse_p
    out = np.mean(np.log1p(np.exp(-np.abs(x))) + np.maximum(x, 0.0))
    return np.float32(out)


# revision 6
# speedup vs baseline: 1.5215x; 1.5215x over previous
"""CircleLoss Trainium2 kernel (8-core SPMD), v4.0.

Math: for S = cosine-sim(enc, dec) [N,N], both loss directions reduce to
per-wrapped-diagonal logsumexps of one matrix:
    out = mean_{d=1..N-1} softplus(L[d] + lse_p)
    L[d]  = log sum_j exp(g(S[j,(j+d)%N])),  g(s) = GAMMA*(max(s,-M)^2 - M^2)
    lse_p = logsumexp_j h(S[j,j])  (exact, computed on host)
g in [-4, 60] so sum(exp(g)) fits f32 with no max-pass.

Per element the device needs E = exp(max(w,-2)^2 - 4) with w = 8*s from a
bf16 matmul (norms and sqrt(GAMMA) folded into host-prepped operands).

Two fused paths, assigned per row-tile to balance engines:
 - B tiles (DVE): one custom DVE op FAUX_EXP_CIRCLE_W computes
       bits = round(sq(max(w,-2)) * 184.665 + 15509.84)  as int16
   straight from PSUM. The int16 written IS the bf16 bit pattern of E
   (Schraudolph): bits(E) ~ 128*(127 + log2 E), log2 E = (u^2-4)*log2(e).
   The constant is calibrated so per-diagonal sums are unbiased to ~0.4%,
   i.e. ~3e-5 on the final loss (gate is 2e-2). Drain+clamp+square+exp in
   one 1x pass, no ACT involvement.
 - C tiles (ACT): Relu(w+2) from PSUM -> t, Square(t-2) -> u^2,
   Exp(u^2 - 4) -> E in bf16. No DVE involvement.

Sharding: core r owns rows [1024r, 1024r+1024). Host pre-normalizes both
embeddings (f64) and ships bf16 transposed operands. Each core computes its
1024 x 8320 sheared slab (row-tile bj reads dec window cols shifted by
128*bj so wrapped diagonals align across tiles: element (p, y) of every
tile has diagonal d = y - p). Row-tiles are summed in PAIRS (one bf16
tensor_tensor add per half, split between DVE and GpSimd), the 4 pair
stripes bounce through DRAM with a sheared re-read that turns diagonals
into columns, and column sums accumulate via one-hot matmuls in PSUM.
Host sums the 8 per-core [8192] partials and finishes in float64.
"""

import numpy as np
import ml_dtypes

import concourse.bass as bass
import concourse.bacc as bacc
import concourse.mybir as mybir
import concourse.dve_ops as dve_ops
from concourse.dve_spec import Spec, Src0, C0, C1, C2, sq, maxx, lower
from concourse.dve_uop import DveOpSpec
from concourse.tile import TileContext
from concourse.bass_utils import run_bass_kernel_spmd

N = 8192
D = 128
P = 128
NCORES = 8
R = N // NCORES          # 1024 rows per core
NBJ = R // P             # 8 row-tiles per core
WIN = 9216               # dec window columns per core
W2 = 8320                # sheared slab width (y = x - 128*bj, d = y - p)
NWC = 16                 # 512-wide d-chunks in the output
NPAIR = 4
M_M = 0.25
GAMMA = 64.0
SQG = 8.0                # sqrt(GAMMA), folded into enc operand on host
EPS = 1e-5

# per-bj groups over y: 5 x 1536 + 1 x 640 (PSUM tile is [P,1536])
GROUPS = [(0, 1536), (1536, 1536), (3072, 1536), (4608, 1536),
          (6144, 1536), (7680, 640)]
# halves for pair-adds / stripe writes; sheared read rq valid after the
# half covering stripe cols [1024rq, 1024rq+1151] is written
HALVES = [(0, 4608, (0, 1, 2, 3)), (4608, 3712, (4, 5, 6, 7))]
TILE_KIND = "BBCBBCBB"           # B = DVE faux path, C = ACT path
ADD_ENGINE = "vgvg"              # per pair: v = DVE, g = GpSimd

# Schraudolph constants: bits = C1F*u^2 + C2F, C1F = 128*log2(e),
# C2F = 16256 - 4*C1F + c_adj, c_adj = -7.5 (centers per-diagonal sums)
C1F = 184.66496414
C2F = 15509.84014

F32 = mybir.dt.float32
F16 = mybir.dt.float16
BF16 = mybir.dt.bfloat16
I16 = mybir.dt.int16

_CACHE = {}


def _register_faux():
    name = "FAUX_EXP_CIRCLE_W_ANT"
    for op in dve_ops.OPS:
        if op.name == name:
            return op
    spec = Spec(
        body=sq(maxx(Src0, C0)) * C1 + C2,
        reference=lambda in0, in1, s0, s1, imm2:
            (np.maximum(in0.astype(np.float32), s0) ** 2 * s1 + imm2
             ).astype(np.float32),
    )
    row = max(dve_ops._SUB_OPCODE_FOR_NAME.values()) + 1
    shas = {}
    for ver in ("v3", "v4"):
        tmp = DveOpSpec(name=name, opcode=row, uops=lower(spec, ver=ver),
                        rd1_en=False)
        shas[ver] = tmp.sha(ver)
    op = dve_ops.DveOp(name, spec, subdim=False, uops_sha=shas)
    dve_ops.OPS.append(op)
    dve_ops._SUB_OPCODE_FOR_NAME[name] = row
    dve_ops.CUSTOM_DVE_SPECS[name] = spec
    return op


FAUX = _register_faux()


def _build_program():
    nc = bacc.Bacc("TRN2", target_bir_lowering=False, debug=False,
                   num_devices=NCORES)
    encT = nc.dram_tensor("encT", [P, R], BF16, kind="ExternalInput")
    decT = nc.dram_tensor("decT", [P, WIN], BF16, kind="ExternalInput")
    acc_out = nc.dram_tensor("acc_out", [NWC, 512], F32, kind="ExternalOutput")
    stripes = nc.dram_tensor("stripes", [NPAIR, P, W2], BF16, kind="Internal")

    add = mybir.AluOpType.add
    AF = mybir.ActivationFunctionType

    with TileContext(nc) as tc:
        with (
            tc.tile_pool(name="persist", bufs=1) as persist,
            tc.tile_pool(name="mm", bufs=2, space="PSUM") as mmp,
            tc.tile_pool(name="apsum", bufs=1, space="PSUM") as apsum,
            tc.tile_pool(name="trow", bufs=2) as trow_pool,
            tc.tile_pool(name="vrow", bufs=1) as vrow_pool,
            tc.tile_pool(name="erow", bufs=2) as erow_pool,
            tc.tile_pool(name="erpool", bufs=4) as erpool,
        ):
            enc_sb = persist.tile([P, R], BF16)
            nc.sync.dma_start(out=enc_sb[:], in_=encT[:, :])
            dec_sb = persist.tile([P, WIN], BF16)
            for dk in range(3):
                nc.sync.dma_start(out=dec_sb[:, dk * 3072:(dk + 1) * 3072],
                                  in_=decT[:, dk * 3072:(dk + 1) * 3072])

            onehot = persist.tile([P, NWC * NWC], BF16)
            bias_p2 = persist.tile([P, 1], F32)
            nc.gpsimd.memset(bias_p2[:], 2.0)
            bias_m2 = persist.tile([P, 1], F32)
            nc.gpsimd.memset(bias_m2[:], -2.0)
            bias_m4 = persist.tile([P, 1], F32)
            nc.gpsimd.memset(bias_m4[:], -4.0)
            nc.gpsimd.memset(onehot[:], 0.0)
            for wc in range(NWC):
                nc.gpsimd.memset(onehot[:, wc * NWC + wc:wc * NWC + wc + 1],
                                 1.0)

            pairs = [persist.tile([P, W2], BF16, name=f"pair{k}")
                     for k in range(NPAIR)]
            acc_ps = apsum.tile([NWC, 512], F32)
            nc.vector.memset(acc_ps[:], 0.0)
            acc_sb = persist.tile([NWC, 512], F32)

            def mm_groups(bj):
                """Yield (g, y0, gw, ps) with the matmul for the group done."""
                for g, (y0, gw) in enumerate(GROUPS):
                    ps = mmp.tile([P, 1536], F32, tag="ps")
                    for c0 in range(0, gw, 512):
                        cw = min(512, gw - c0)
                        nc.tensor.matmul(
                            ps[:, c0:c0 + cw],
                            lhsT=enc_sb[:, bj * P:(bj + 1) * P],
                            rhs=dec_sb[:, 128 * bj + y0 + c0:
                                       128 * bj + y0 + c0 + cw],
                            start=True, stop=True)
                    yield g, y0, gw, ps

            def compute_tile(bj, dst):
                """Fill dst[P, W2] (bf16 tile) with E values for row-tile bj."""
                if TILE_KIND[bj] == "B":
                    dst_i16 = dst[:].bitcast(I16)
                    for g, y0, gw, ps in mm_groups(bj):
                        nc.vector._custom_dve(
                            FAUX, out=dst_i16[:, y0:y0 + gw], in0=ps[:, 0:gw],
                            s0=-2.0, s1=C1F, imm2=C2F)
                else:
                    tr = trow_pool.tile([P, W2], F16, tag="tr")
                    vr = vrow_pool.tile([P, W2], F16, tag="vr")
                    for g, y0, gw, ps in mm_groups(bj):
                        nc.scalar.activation(tr[:, y0:y0 + gw], ps[:, 0:gw],
                                             AF.Relu, bias=bias_p2[:, 0:1], scale=1.0)
                    for h0, hw, _ in HALVES:
                        nc.scalar.activation(vr[:, h0:h0 + hw],
                                             tr[:, h0:h0 + hw], AF.Square,
                                             bias=bias_m2[:, 0:1], scale=1.0)
                        nc.scalar.activation(dst[:, h0:h0 + hw],
                                             vr[:, h0:h0 + hw], AF.Exp,
                                             bias=bias_m4[:, 0:1], scale=1.0)

            for k in range(NPAIR):
                compute_tile(2 * k, pairs[k])
                er_ = erow_pool.tile([P, W2], BF16, tag="erow")
                compute_tile(2 * k + 1, er_)
                eng = nc.vector if ADD_ENGINE[k] == "v" else nc.gpsimd
                for h0, hw, rqs in HALVES:
                    eng.tensor_tensor(out=pairs[k][:, h0:h0 + hw],
                                      in0=er_[:, h0:h0 + hw],
                                      in1=pairs[k][:, h0:h0 + hw], op=add)
                    nc.sync.dma_start(out=stripes[k, :, h0:h0 + hw],
                                      in_=pairs[k][:, h0:h0 + hw])
                    for rq in rqs:
                        er2 = erpool.tile([P, 1024], BF16, tag="er")
                        nc.sync.dma_start(
                            out=er2[:],
                            in_=bass.AP(tensor=stripes,
                                        offset=k * P * W2 + 1024 * rq,
                                        ap=[[W2 + 1, P], [1, 1024]]))
                        for h in range(2):
                            wc = 2 * rq + h
                            nc.tensor.matmul(
                                acc_ps[:],
                                lhsT=onehot[:, wc * NWC:(wc + 1) * NWC],
                                rhs=er2[:, h * 512:(h + 1) * 512],
                                start=False, stop=False,
                                skip_group_check=True)
            nc.scalar.copy(acc_sb[:], acc_ps[:])
            nc.sync.dma_start(out=acc_out[:, :], in_=acc_sb[:])
    nc.compile()
    return nc


def make_in_maps(enc: np.ndarray, dec: np.ndarray):
    """Host prep: normalize in f64, fold sqrt(GAMMA) into enc, transpose,
    cast bf16, build per-core window slices. Returns (in_maps, lse_p)."""
    e64 = enc.astype(np.float64)
    d64 = dec.astype(np.float64)
    en = np.sqrt((e64 * e64).sum(1, keepdims=True))
    dn = np.sqrt((d64 * d64).sum(1, keepdims=True))
    encn8 = (e64 / en * SQG).astype(ml_dtypes.bfloat16)
    decn = (d64 / dn).astype(ml_dtypes.bfloat16)

    s_jj = (e64 * d64).sum(1) / (en[:, 0] * dn[:, 0] + EPS)
    h = -np.maximum(1.0 + M_M - s_jj, 0.0) * (s_jj - (1.0 - M_M)) * GAMMA
    hm = h.max()
    lse_p = hm + np.log(np.exp(h - hm).sum())

    in_maps = []
    for r in range(NCORES):
        idx = (r * R + np.arange(WIN)) % N
        in_maps.append({
            "encT": np.ascontiguousarray(encn8[r * R:(r + 1) * R].T),
            "decT": np.ascontiguousarray(decn[idx].T),
        })
    return in_maps, lse_p


def kernel(encoder_output: np.ndarray, decoder_output: np.ndarray) -> np.ndarray:
    enc = np.ascontiguousarray(encoder_output, dtype=np.float32)
    dec = np.ascontiguousarray(decoder_output, dtype=np.float32)
    assert enc.shape == (N, D) and dec.shape == (N, D)

    if "nc" not in _CACHE:
        _CACHE["nc"] = _build_program()
    nc = _CACHE["nc"]

    in_maps, lse_p = make_in_maps(enc, dec)
    res = run_bass_kernel_spmd(nc, in_maps, core_ids=list(range(NCORES)))

    sum_exp = np.zeros(N, dtype=np.float64)
    for r in range(NCORES):
        acc = res.results[r]["acc_out"].astype(np.float64)      # [NWC, 512]
        sum_exp += acc.reshape(N)                               # d = 512*wc + f
    L = np.log(sum_exp[1:])
    x = L + lse_p
    out = np.mean(np.log1p(np.exp(-np.abs(x))) + np.maximum(x, 0.0))
    return np.float32(out)
